# revision 46
# baseline (speedup 1.0000x reference)
"""Trainium2 Bass kernel for nn_G3DCrossAttention (B=2, C=512, L=2048, G=2048, H=8).

Math (exact rank-1 collapse of the cross-attention):
  exp_p[g,b,:] = exp[b,g]*Wg[:,0] + bg  =>  k/v are rank-1 in channel dim;
  softmax collapses to w_i = f_b(a_i) with a = x_seq^T M + a0 (per head),
  f_b evaluated exactly at 64 Chebyshev nodes per batch, fit with a
  degree-KDEG Chebyshev series, evaluated by Clenshaw.
  x_attn = w*u_v + c_v per head; then LN1 / FFN / LN2 / Wo as usual.

v3 schedule (from v2 trace): node stage fully front-loaded (e_b via DMA
broadcast, pn first on Act queue, ck round-trip issued early), stage-A
de-hopped (psum-direct scalar reads, combined uv/vbg transpose), per-batch
Clenshaw chains split across DVE and Pool, LN row chain shortened via a
fused (stat1+eps)-mu^2 st_t, weight DMAs ordered by first use across the
three queues (SP/Act/Pool).
"""

from contextlib import ExitStack

import ml_dtypes
import numpy as np

import concourse.bass as bass
import concourse.tile as tile
from concourse import bacc, mybir
from concourse.bass_utils import run_bass_kernel_spmd

F32 = mybir.dt.float32
F32R = mybir.dt.float32r
FP16 = mybir.dt.float16
AF = mybir.ActivationFunctionType
OP = mybir.AluOpType

B, C, L, G, H = 2, 512, 2048, 2048, 8
D = C // H
NCORES = 8
LC = L // NCORES              # 256 queries per core
T = B * LC                    # 512 tokens per core (tau = b*LC + l)
KC = C // 128                 # 4 partition tiles over C
KH = (4 * C) // 128           # 16 partition tiles over 4C
SCALE = 1.0 / float(np.sqrt(D))
EPS = 1e-5
SCAL = 5.0                    # Chebyshev half-range in a-units (|a|max ~ 4.43)
KDEG = 12                     # Chebyshev series length (max err ~7e-3)
MNODES = 64                   # Chebyshev nodes per batch (2 batches -> 128 parts)
SS = SCALE / SCAL
IDF = 2 * KDEG + 1            # f32 identity offset inside constB
CLENSHAW_POOL = False          # batch-1 Clenshaw chain on GpSimd (Pool)

TRACE = False
TRACE_KW = {}
LAST_RESULTS = None

_CACHE = None


def _consts():
    m = np.arange(MNODES)
    theta = np.pi * (2 * m + 1) / (2 * MNODES)
    xn64 = (SCAL * np.cos(theta)).astype(np.float32)
    xnodes = np.concatenate([xn64, xn64])                 # [128] both batches
    dct1 = np.zeros((MNODES, KDEG), np.float32)
    for k in range(KDEG):
        dct1[:, k] = (2.0 / MNODES) * np.cos(k * theta)
    dct1[:, 0] *= 0.5
    dctbd = np.zeros((2 * MNODES, 2 * KDEG), np.float32)  # block-diag [128, 2K]
    dctbd[:MNODES, :KDEG] = dct1
    dctbd[MNODES:, KDEG:] = dct1
    # constA fp16 [128, 1]: ones/C column (LN stats stationary)
    constA = np.full((128, 1), 1.0 / C, np.float16)
    # constB f32: block-diag DCT | cheb nodes | f32 id | selKb | diagmask
    constB = np.zeros((128, IDF + 256 + KDEG), np.float32)
    constB[:, 0:2 * KDEG] = dctbd
    constB[:, 2 * KDEG] = xnodes
    constB[:, IDF:IDF + 128] = np.eye(128, dtype=np.float32)
    for r in range(2 * KDEG):
        for p in range(128):
            if r // KDEG == p // 64:
                constB[r, IDF + 128 + p] = 1.0
        constB[r, IDF + 256 + (r % KDEG)] = 1.0
    # constC fp16 [2, 640]: sel/halfs blocks | ones row
    constC = np.zeros((2, 640), np.float16)
    constC[0, 0:64] = 1.0
    constC[1, 64:128] = 1.0
    constC[:, 128:640] = 1.0
    return constA, constB, constC


def _build():
    nc = bacc.Bacc(debug=False, num_devices=NCORES)

    # ---- external inputs -------------------------------------------------
    seq_sl = nc.dram_tensor("seq_sl", [B, C, LC], FP16, kind="ExternalInput")
    exp16 = nc.dram_tensor("exp16", [B, G], FP16, kind="ExternalInput")
    wqt16 = nc.dram_tensor("wqt16", [C, C], FP16, kind="ExternalInput")     # Wq.T
    wot = nc.dram_tensor("wot", [C, C], FP16, kind="ExternalInput")         # (Wo*g2).T
    ut16d = nc.dram_tensor("ut16", [C, H], FP16, kind="ExternalInput")      # mask*uk
    us9d = nc.dram_tensor("us9d", [KC * (H + 1), 128], FP16, kind="ExternalInput")
    a0bd = nc.dram_tensor("a0bd", [128, H], FP16, kind="ExternalInput")     # SS*a0 bc
    w1t = nc.dram_tensor("w1t", [C, 4 * C], FP16, kind="ExternalInput")     # (W1*g1).T
    w2t = nc.dram_tensor("w2t", [4 * C, C], FP16, kind="ExternalInput")     # W2.T
    smallsf = nc.dram_tensor("smallsf", [32, 128], F32, kind="ExternalInput")

    out_sl = nc.dram_tensor("out_sl", [B, C, LC], F32, kind="ExternalOutput")

    constA_np, constB_np, constC_np = _consts()
    c_A = nc.inline_tensor(constA_np, name="c_A")
    c_B = nc.inline_tensor(constB_np, name="c_B")
    c_C = nc.inline_tensor(constC_np, name="c_C")

    with tile.TileContext(nc) as tc, ExitStack() as ctx:
        p_w1 = ctx.enter_context(tc.tile_pool(name="w1", bufs=KC))
        p_w2 = ctx.enter_context(tc.tile_pool(name="w2", bufs=16))
        p_kvh = ctx.enter_context(tc.tile_pool(name="kvh", bufs=8))
        p_wo = ctx.enter_context(tc.tile_pool(name="wo", bufs=KC))
        p_xsz = ctx.enter_context(tc.tile_pool(name="xsz", bufs=4))
        p_act = ctx.enter_context(tc.tile_pool(name="act", bufs=4))
        p_node = ctx.enter_context(tc.tile_pool(name="node", bufs=1))
        p_sm = ctx.enter_context(tc.tile_pool(name="sm", bufs=1))
        p_cl = ctx.enter_context(tc.tile_pool(name="cl", bufs=1))
        ps_mm = ctx.enter_context(tc.tile_pool(name="psmm", bufs=5, space="PSUM"))
        ps_x = ctx.enter_context(tc.tile_pool(name="psx", bufs=1, space="PSUM"))

        # ============ DMA queue programs (order == queue order) ===========
        # sync (SP): consts, stage-A host vectors, wqt, wot; w2 at the tail
        cC_sb = p_sm.tile([2, 640], FP16, tag="cC")
        nc.sync.dma_start(cC_sb[:], c_C[:])
        cA_sb = p_sm.tile([128, 1], FP16, tag="cA")
        nc.sync.dma_start(cA_sb[:], c_A[:])
        u_t = [p_sm.tile([128, H], FP16, tag=f"u{kt}", name=f"u_{kt}")
               for kt in range(KC)]
        for kt in range(KC):
            nc.sync.dma_start(u_t[kt][:], ut16d[kt * 128:(kt + 1) * 128, :])
        usel9 = [p_sm.tile([H + 1, 128], FP16, tag=f"us{kt}", name=f"us_{kt}")
                 for kt in range(KC)]
        for kt in range(KC):
            nc.sync.dma_start(usel9[kt][:],
                              us9d[kt * (H + 1):(kt + 1) * (H + 1), :])
        a0b = p_sm.tile([128, H], FP16, tag="a0b")
        nc.sync.dma_start(a0b[:], a0bd[:])
        wo_t = [p_wo.tile([128, C], FP16, tag="wo", name=f"wo_{i}")
                for i in range(KC)]
        for kt in range(KC):
            nc.sync.dma_start(wo_t[kt][:], wot[kt * 128:(kt + 1) * 128, :])

        # scalar (Act): exp only; bulk stays off this queue
        exp_sb = p_sm.tile([2, G], FP16, tag="expsb")
        nc.scalar.dma_start(exp_sb[:], exp16[:])

        # gpsimd (Pool): consts, xs, w1
        cB_sb = p_sm.tile([128, IDF + 256 + KDEG], F32, tag="cB")
        nc.gpsimd.dma_start(cB_sb[:], c_B[:])
        smf_sb = p_sm.tile([32, 128], F32, tag="smf")
        nc.gpsimd.dma_start(smf_sb[:], smallsf[:])
        xs_t = [p_xsz.tile([128, T], FP16, tag="xs", name=f"xs_{i}")
                for i in range(KC)]
        wqt_t = [p_kvh.tile([128, C], FP16, tag="kv", name=f"wqt_{i}")
                 for i in range(KC)]
        for kt in range(KC):
            srcx = seq_sl[:, kt * 128:(kt + 1) * 128, :].rearrange("b c l -> c b l")
            nc.gpsimd.dma_start(xs_t[kt][:], srcx)
            nc.gpsimd.dma_start(wqt_t[kt][:], wqt16[kt * 128:(kt + 1) * 128, :])
        w1_t = [p_w1.tile([128, 4 * C], FP16, tag="w1", name=f"w1_{i}")
                for i in range(KC)]
        for kt in range(KC):
            nc.gpsimd.dma_start(w1_t[kt][:], w1t[kt * 128:(kt + 1) * 128, :])

        eps_col = p_sm.tile([1, 1], F32, tag="epsc")
        nc.vector.memset(eps_col[:], EPS)

        # ============ PE: colsf transpose =================================
        colsf_ps = ps_x.tile([128, 32], F32, tag="small", bufs=1, name="colsfps")
        nc.tensor.transpose(colsf_ps[:], smf_sb[:], cB_sb[0:32, IDF:IDF + 32])
        colsf = p_sm.tile([128, 32], F32, tag="colsf")
        nc.vector.tensor_copy(colsf[:], colsf_ps[:])
        # colsf cols: bv 0:4 | g1 4:8 | b1p 8:24 | b2p 24:28 | bop 28:32

        # ============ PE: exp broadcast to [128, G] =======================
        eb_ps = [ps_mm.tile([128, 512], F32, tag="mm", name=f"ebps{j}")
                 for j in range(4)]
        for j in range(4):
            nc.tensor.matmul(eb_ps[j][:], cC_sb[:, 0:128],
                             exp_sb[:, j * 512:(j + 1) * 512],
                             start=True, stop=True)
        eb16 = p_node.tile([128, G], FP16, tag="eb16")
        for j in range(4):
            nc.scalar.copy(eb16[:, j * 512:(j + 1) * 512], eb_ps[j][:])

        # ============ node stage on Act: pn = exp(xn*e), accum z ==========
        pn = p_node.tile([128, G], F32, tag="pn")
        z4 = p_sm.tile([128, 4], F32, tag="z4")
        nm4 = p_sm.tile([128, 4], F32, tag="nm4")
        for j in range(4):
            nc.scalar.activation(pn[:, j * 512:(j + 1) * 512],
                                 eb16[:, j * 512:(j + 1) * 512], AF.Exp,
                                 scale=cB_sb[:, 2 * KDEG:2 * KDEG + 1],
                                 accum_out=z4[:, j:j + 1])

        # ============ DVE node tail: weighted sum, f ======================
        for j in range(4):
            nc.vector.scalar_tensor_tensor(
                out=pn[:, j * 512:(j + 1) * 512],
                in0=pn[:, j * 512:(j + 1) * 512], scalar=1.0,
                in1=eb16[:, j * 512:(j + 1) * 512],
                op0=OP.mult, op1=OP.mult, accum_out=nm4[:, j:j + 1])
        z_col = p_sm.tile([128, 1], F32, tag="zc")
        nc.vector.tensor_reduce(z_col[:], z4[:], axis=mybir.AxisListType.X, op=OP.add)
        nm_col = p_sm.tile([128, 1], F32, tag="nmc")
        nc.vector.tensor_reduce(nm_col[:], nm4[:], axis=mybir.AxisListType.X, op=OP.add)
        zr_col = p_sm.tile([128, 1], F32, tag="zrc")
        nc.vector.reciprocal(zr_col[:], z_col[:])
        f_col = p_sm.tile([128, 1], F32, tag="fc")
        nc.vector.tensor_mul(f_col[:], nm_col[:], zr_col[:])

        # ============ q = Wq.T-tiles @ xs (no M precompute) ===============
        # q[m, tau] = sum_c Wq[m, c] xs[c, tau]; stationary = Wq.T tiles
        q16 = []
        for mt in range(KC):
            q_ps = ps_mm.tile([128, T], F32, tag="mm", name=f"qps{mt}")
            for kt in range(KC):
                nc.tensor.matmul(q_ps[:], wqt_t[kt][:, mt * 128:(mt + 1) * 128],
                                 xs_t[kt][:], start=(kt == 0), stop=(kt == KC - 1))
            qm = p_act.tile([128, T], FP16, tag="q", name=f"q{mt}")
            nc.scalar.copy(qm[:], q_ps[:])
            q16.append(qm)
        # ============ coeffs: DCT matmul + DRAM round trip ================
        pck = ps_x.tile([2 * KDEG, 1], F32, tag="small", bufs=1, name="pck")
        nc.tensor.matmul(pck[:], cB_sb[:, 0:2 * KDEG], f_col[:],
                         start=True, stop=True)
        ck_sb = p_sm.tile([2 * KDEG, 1], F32, tag="cksb")
        nc.vector.tensor_copy(ck_sb[:], pck[:])
        ckdiag = p_sm.tile([2 * KDEG, KDEG], F32, tag="ckdiag")
        nc.vector.tensor_scalar_mul(
            ckdiag[:], cB_sb[0:2 * KDEG, IDF + 256:IDF + 256 + KDEG], ck_sb[:])
        cb_ps = ps_x.tile([128, KDEG], F32, tag="small", bufs=1, name="cbps")
        nc.tensor.matmul(cb_ps[:], cB_sb[0:2 * KDEG, IDF + 128:IDF + 256],
                         ckdiag[:], start=True, stop=True)
        cb = p_cl.tile([128, KDEG], F32, tag="cb")
        nc.vector.tensor_copy(cb[:], cb_ps[:])

        # ====== a.T packed [p = b*64+t64, (blk, h)] from q16 ==============
        FPK = 4 * (H + 1)                                 # 36 free cols
        HB = FPK // 2
        tt = p_cl.tile([128, FPK], F32, tag="tt")
        nc.gpsimd.memset(tt[:], 0.0)
        pa_all = ps_x.tile([128, 4 * H], F32, tag="st1", name="paall")
        for b in range(B):
            for blk in range(4):
                tsl = slice(b * 256 + blk * 64, b * 256 + (blk + 1) * 64)
                out = pa_all[b * 64:(b + 1) * 64, blk * 8:(blk + 1) * 8]
                for mt in range(KC):
                    nc.tensor.matmul(out, q16[mt][:, tsl], u_t[mt][:],
                                     start=(mt == 0), stop=(mt == KC - 1))
        for b in range(B):
            for blk in range(4):
                nc.vector.scalar_tensor_tensor(
                    out=tt[b * 64:(b + 1) * 64, blk * 9:blk * 9 + H],
                    in0=pa_all[b * 64:(b + 1) * 64, blk * 8:(blk + 1) * 8],
                    scalar=SS,
                    in1=a0b[b * 64:(b + 1) * 64, :],
                    op0=OP.mult, op1=OP.add)

        # ====== per-batch clamp + Clenshaw (DVE), unpack, y ===============
        tt2 = p_cl.tile([128, FPK], F32, tag="tt2")
        bb1 = p_cl.tile([128, FPK], F32, tag="bb1")
        bb2 = p_cl.tile([128, FPK], F32, tag="bb2")
        tmp = p_cl.tile([128, FPK], F32, tag="tmp")
        w_c = p_cl.tile([128, FPK], F32, tag="wp")
        w_rows = p_sm.tile([H + 1, T], FP16, tag="wrows")

        def clenshaw_full():
            nc.vector.tensor_scalar_max(tt[:], tt[:], -1.0)
            nc.vector.tensor_scalar_min(tt[:], tt[:], 1.0)
            nc.vector.tensor_add(tt2[:], tt[:], tt[:])
            cur1, cur2 = bb1, bb2
            for k in range(KDEG - 1, 0, -1):
                nc.vector.tensor_mul(tmp[:], tt2[:], cur1[:])
                nc.vector.scalar_tensor_tensor(
                    out=cur2[:], in0=tmp[:], scalar=cb[:, k:k + 1],
                    in1=cur2[:], op0=OP.add, op1=OP.subtract)
                cur1, cur2 = cur2, cur1
            nc.vector.tensor_mul(tmp[:], tt[:], cur1[:])
            nc.vector.scalar_tensor_tensor(
                out=w_c[:], in0=tmp[:], scalar=cb[:, 0:1],
                in1=cur2[:], op0=OP.add, op1=OP.subtract)
            for tb in range(4):
                nc.gpsimd.memset(w_c[:, tb * 9 + H:tb * 9 + H + 1], 1.0)

        def unpack_half(b):
            idsl = cB_sb[b * 64:(b + 1) * 64, IDF + b * 64:IDF + (b + 1) * 64]
            for blk in range(4):
                wr_ps = ps_x.tile([H + 1, 64], F32,
                                  tag="small" if blk % 2 == 0 else "st1",
                                  bufs=1, name=f"wrps{b}{blk}")
                nc.tensor.transpose(
                    wr_ps[:], w_c[b * 64:(b + 1) * 64, blk * 9:(blk + 1) * 9],
                    idsl)
                tsl = slice(b * 256 + blk * 64, b * 256 + (blk + 1) * 64)
                nc.scalar.copy(w_rows[0:H + 1, tsl], wr_ps[:])

        y_t = [p_act.tile([128, T], FP16, tag="y", bufs=8, name=f"y{kt}")
               for kt in range(KC)]

        def ymm_half(hf):
            hsl = slice(hf * 256, (hf + 1) * 256)
            for kt in range(KC):
                xa_ps = ps_mm.tile([128, 256], F32, tag="mm", name=f"xa{hf}{kt}")
                nc.tensor.matmul(xa_ps[:], usel9[kt][:], w_rows[:, hsl],
                                 start=True, stop=True)
                nc.vector.tensor_add(y_t[kt][:, hsl], xa_ps[:], xs_t[kt][:, hsl])

        # ==== token-half pipelined LN1 -> FFN1 -> FFN2 -> LN2 -> Wo -> out ====
        # Each half (256 tokens) flows independently; PE queue interleaves
        # halves so LN row-chains overlap the other half's matmuls.
        HT = T // 2

        def ln_stats(y_tiles, hf, ph):
            hsl = slice(hf * HT, (hf + 1) * HT)
            stat0 = ps_x.tile([1, HT], F32, tag="st0", name=f"st0{ph}{hf}")
            for kt in range(KC):
                nc.tensor.matmul(stat0[:], cA_sb[:, 0:1], y_tiles[kt][:, hsl],
                                 start=(kt == 0), stop=(kt == KC - 1))
            sq_t = []
            for kt in range(KC):
                sq = p_act.tile([128, HT], FP16, tag="sq", bufs=4,
                                name=f"sq{ph}{hf}{kt}")
                if kt % 2 == 0:
                    nc.scalar.activation(sq[:], y_tiles[kt][:, hsl], AF.Square)
                else:
                    nc.vector.tensor_mul(sq[:], y_tiles[kt][:, hsl],
                                         y_tiles[kt][:, hsl])
                sq_t.append(sq)
            musq = p_sm.tile([1, HT], F32, tag="lnrow", bufs=8, name=f"mu{ph}{hf}")
            nc.scalar.activation(musq[:], stat0[:], AF.Square)
            stat1 = ps_x.tile([1, HT], F32, tag="st1", name=f"st1{ph}{hf}")
            for kt in range(KC):
                nc.tensor.matmul(stat1[:], cA_sb[:, 0:1], sq_t[kt][:],
                                 start=(kt == 0), stop=(kt == KC - 1))
            vare = p_sm.tile([1, HT], F32, tag="lnrow", bufs=8, name=f"va{ph}{hf}")
            nc.vector.scalar_tensor_tensor(
                out=vare[:], in0=stat1[:], scalar=EPS, in1=musq[:],
                op0=OP.add, op1=OP.subtract)
            varr = p_sm.tile([1, HT], F32, tag="lnrow", bufs=8, name=f"vr{ph}{hf}")
            nc.vector.reciprocal(varr[:], vare[:])
            rstd_row = p_sm.tile([1, HT], FP16, tag="rstdr", bufs=4,
                                 name=f"rs{ph}{hf}")
            with nc.allow_low_precision(reason="fp16 feeds full-rate PE matmul"):
                nc.scalar.activation(rstd_row[:], varr[:], AF.Sqrt)
            q_row = p_sm.tile([1, HT], FP16, tag="qr", bufs=4, name=f"qq{ph}{hf}")
            nc.vector.tensor_mul(q_row[:], stat0[:], rstd_row[:])
            return rstd_row, q_row

        def ln_apply(y_tiles, rstd_row, q_row, out_tiles, hf, ph):
            hsl = slice(hf * HT, (hf + 1) * HT)
            pA = ps_mm.tile([128, HT], F32, tag="mm", name=f"pA{ph}{hf}")
            nc.tensor.matmul(pA[:], cC_sb[0:1, 128:256], rstd_row[:],
                             start=True, stop=True)
            pB = ps_mm.tile([128, HT], F32, tag="mm", name=f"pB{ph}{hf}")
            nc.tensor.matmul(pB[:], cC_sb[0:1, 128:256], q_row[:],
                             start=True, stop=True)
            for kt in range(KC):
                tx = p_act.tile([128, HT], FP16, tag="tmpx", bufs=4,
                                name=f"tx{ph}{hf}{kt}")
                eng = nc.vector if kt % 2 == 0 else nc.gpsimd
                nc.vector.tensor_mul(tx[:], y_tiles[kt][:, hsl], pA[:])
                nc.vector.tensor_sub(out_tiles[kt][:, hsl], tx[:], pB[:])

        n1_t = [p_act.tile([128, T], FP16, tag="x", name=f"n1{kt}")
                for kt in range(KC)]
        h_t = [p_kvh.tile([128, T], FP16, tag="h", bufs=16, name=f"h{mt}")
               for mt in range(KH)]
        y2_t = [p_act.tile([128, T], FP16, tag="y", bufs=8, name=f"y2{mt}")
                for mt in range(KC)]
        z_t = [p_xsz.tile([128, T], FP16, tag="z", name=f"z{mt}")
               for mt in range(KC)]

        def ffn1(hf):
            hsl = slice(hf * HT, (hf + 1) * HT)
            for mt in range(KH):
                sl = slice(mt * 128, (mt + 1) * 128)
                pf = ps_mm.tile([128, HT], F32, tag="mm", name=f"pf1{hf}{mt}")
                for kt in range(KC):
                    nc.tensor.matmul(pf[:], w1_t[kt][:, sl], n1_t[kt][:, hsl],
                                     start=(kt == 0), stop=(kt == KC - 1))
                if mt % 2 == 0:
                    nc.scalar.activation(h_t[mt][:, hsl], pf[:], AF.Relu,
                                         bias=colsf[:, 8 + mt:9 + mt])
                else:
                    nc.vector.tensor_scalar(h_t[mt][:, hsl], pf[:],
                                            colsf[:, 8 + mt:9 + mt],
                                            0.0, op0=OP.add, op1=OP.max)

        def ffn2(hf):
            hsl = slice(hf * HT, (hf + 1) * HT)
            for mt in range(KC):
                sl = slice(mt * 128, (mt + 1) * 128)
                pf = ps_mm.tile([128, HT], F32, tag="mm", name=f"pf2{hf}{mt}")
                for kt in range(KH):
                    nc.tensor.matmul(pf[:], w2_t[kt][:, sl], h_t[kt][:, hsl],
                                     start=(kt == 0), stop=(kt == KH - 1))
                tmp2 = p_act.tile([128, HT], FP16, tag="tmpx", bufs=4,
                                  name=f"t2{hf}{mt}")
                nc.scalar.activation(tmp2[:], pf[:], AF.Identity,
                                     bias=colsf[:, 24 + mt:25 + mt])
                nc.vector.scalar_tensor_tensor(
                    out=y2_t[mt][:, hsl], in0=n1_t[mt][:, hsl],
                    scalar=colsf[:, 4 + mt:5 + mt],
                    in1=tmp2[:], op0=OP.mult, op1=OP.add)

        def wo_out(hf):
            hsl = slice(hf * HT, (hf + 1) * HT)
            for mt in range(KC):
                sl = slice(mt * 128, (mt + 1) * 128)
                pf = ps_mm.tile([128, HT], F32, tag="mm", name=f"pfo{hf}{mt}")
                for kt in range(KC):
                    nc.tensor.matmul(pf[:], wo_t[kt][:, sl], z_t[kt][:, hsl],
                                     start=(kt == 0), stop=(kt == KC - 1))
                om = p_act.tile([128, HT], F32, tag="om", bufs=4,
                                name=f"om{hf}{mt}")
                nc.scalar.activation(om[:], pf[:], AF.Identity,
                                     bias=colsf[:, 28 + mt:29 + mt])
                # half hf covers tokens [hf*256, hf*256+256) = batch hf entirely
                for lh in range(2):
                    eng = (nc.scalar, nc.sync, nc.gpsimd)[(2 * mt + lh) % 3]
                    eng.dma_start(
                        out_sl[hf, mt * 128:(mt + 1) * 128,
                               lh * 128:(lh + 1) * 128],
                        om[:, lh * 128:(lh + 1) * 128])

        # w2 loads (tail of sync queue; emitted late for sem-slot hygiene)
        w2_t = [p_w2.tile([128, C], FP16, tag="w2", name=f"w2_{i}")
                for i in range(KH)]
        for kt in range(KH):
            nc.sync.dma_start(w2_t[kt][:], w2t[kt * 128:(kt + 1) * 128, :])

        # interleaved schedule: batch-0 w-path flows while batch-1 Clenshaw
        # runs on DVE; halves alternate so LN row-chains hide under matmuls
        nc.gpsimd.memset(bb1[:], 0.0)
        nc.gpsimd.memset(bb2[:], 0.0)
        clenshaw_full()
        unpack_half(0)
        ymm_half(0)
        r0, q0 = ln_stats(y_t, 0, "a")
        unpack_half(1)
        ymm_half(1)
        r1, q1 = ln_stats(y_t, 1, "a")
        ln_apply(y_t, r0, q0, n1_t, 0, "a")
        ffn1(0)
        ln_apply(y_t, r1, q1, n1_t, 1, "a")
        ffn1(1)
        ffn2(0)
        r20, q20 = ln_stats(y2_t, 0, "b")
        ffn2(1)
        ln_apply(y2_t, r20, q20, z_t, 0, "b")
        r21, q21 = ln_stats(y2_t, 1, "b")
        wo_out(0)
        ln_apply(y2_t, r21, q21, z_t, 1, "b")
        wo_out(1)

    nc.compile()
    return nc


def kernel(**inputs):
    global _CACHE, LAST_RESULTS
    if _CACHE is None:
        _CACHE = _build()
    nc = _CACHE

    f32 = lambda x: np.ascontiguousarray(np.asarray(x), dtype=np.float32)
    f16 = lambda x: np.ascontiguousarray(np.asarray(x), dtype=np.float16)
    seq = f16(inputs["seq"])
    W1 = np.asarray(inputs["W1"], np.float32)
    W2 = np.asarray(inputs["W2"], np.float32)
    Wo = np.asarray(inputs["Wo"], np.float32)
    g1 = np.asarray(inputs["g1"], np.float32)
    g2 = np.asarray(inputs["g2"], np.float32)
    beta1 = np.asarray(inputs["beta1"], np.float32)
    beta2 = np.asarray(inputs["beta2"], np.float32)
    b1p = np.asarray(inputs["b1"], np.float32) + W1 @ beta1
    b2p = np.asarray(inputs["b2"], np.float32) + beta1
    bop = np.asarray(inputs["bo"], np.float32) + Wo @ beta2
    Wg = np.asarray(inputs["Wg"], np.float32)
    bg = np.asarray(inputs["bg"], np.float32)
    bq = np.asarray(inputs["bq"], np.float32)
    bv = np.asarray(inputs["bv"], np.float32)

    # stage-A vectors (token-independent, exact in fp32 on host)
    Wk = np.asarray(inputs["Wk"], np.float32)
    Wv = np.asarray(inputs["Wv"], np.float32)
    wg0 = Wg[:, 0]
    uk = Wk @ wg0                                     # [C]
    uv = Wv @ wg0                                     # [C]
    cv = Wv @ bg + bv                                 # [C]
    mask = np.zeros((C, H), np.float32)
    for h in range(H):
        mask[h * (C // H):(h + 1) * (C // H), h] = 1.0
    ut = mask * uk[:, None]                           # [C, H]
    a0 = ut.T @ bq                                    # [H]
    us9 = np.zeros((KC * (H + 1), 128), np.float32)
    for kt in range(KC):
        us9[kt * (H + 1):kt * (H + 1) + H, :] = \
            (mask[kt * 128:(kt + 1) * 128, :] * uv[kt * 128:(kt + 1) * 128, None]).T
        us9[kt * (H + 1) + H, :] = cv[kt * 128:(kt + 1) * 128]
    a0b = np.broadcast_to((SS * a0)[None, :], (128, H)).copy()
    smallsf = np.zeros((32, 128), np.float32)
    smallsf[0:4] = bv.reshape(4, 128)
    smallsf[4:8] = g1.reshape(4, 128)
    smallsf[8:24] = b1p.reshape(16, 128)
    smallsf[24:28] = b2p.reshape(4, 128)
    smallsf[28:32] = bop.reshape(4, 128)

    base = {
        "exp16": f16(inputs["exp"]),
        "wqt16": f16(np.asarray(inputs["Wq"]).T),
        "wot": f16((Wo * g2[None, :]).T),
        "ut16": f16(ut),
        "us9d": f16(us9),
        "a0bd": f16(a0b),
        "w1t": f16((W1 * g1[None, :]).T),
        "w2t": f16(W2.T),
        "smallsf": smallsf,
    }
    in_maps = []
    for c in range(NCORES):
        m = dict(base)
        m["seq_sl"] = np.ascontiguousarray(seq[:, :, c * LC:(c + 1) * LC])
        in_maps.append(m)

    res = run_bass_kernel_spmd(nc, in_maps, list(range(NCORES)), trace=TRACE,
                               **TRACE_KW)
    LAST_RESULTS = res
    out = np.empty((B, C, L), np.float32)
    for c in range(NCORES):
        out[:, :, c * LC:(c + 1) * LC] = res.results[c]["out_sl"]
    return out


# revision 47
# speedup vs baseline: 1.1804x; 1.1804x over previous
"""Trainium2 Bass kernel for nn_G3DCrossAttention (B=2, C=512, L=2048, G=2048, H=8).

Math (exact rank-1 collapse of the cross-attention):
  exp_p[g,b,:] = exp[b,g]*Wg[:,0] + bg  =>  k/v are rank-1 in channel dim;
  softmax collapses to w_i = f_b(a_i) with a = x_seq^T M + a0 (per head),
  f_b evaluated exactly at 64 Chebyshev nodes per batch, fit with a
  degree-KDEG Chebyshev series, evaluated by Clenshaw.
  x_attn = w*u_v + c_v per head; then LN1 / FFN / LN2 / Wo as usual.

v3 schedule (from v2 trace): node stage fully front-loaded (e_b via DMA
broadcast, pn first on Act queue, ck round-trip issued early), stage-A
de-hopped (psum-direct scalar reads, combined uv/vbg transpose), per-batch
Clenshaw chains split across DVE and Pool, LN row chain shortened via a
fused (stat1+eps)-mu^2 st_t, weight DMAs ordered by first use across the
three queues (SP/Act/Pool).
"""

from contextlib import ExitStack

import ml_dtypes
import numpy as np

import concourse.bass as bass
import concourse.tile as tile
from concourse import bacc, mybir
from concourse.bass_utils import run_bass_kernel_spmd

F32 = mybir.dt.float32
F32R = mybir.dt.float32r
FP16 = mybir.dt.float16
AF = mybir.ActivationFunctionType
OP = mybir.AluOpType

B, C, L, G, H = 2, 512, 2048, 2048, 8
D = C // H
NCORES = 8
LC = L // NCORES              # 256 queries per core
T = B * LC                    # 512 tokens per core (tau = b*LC + l)
KC = C // 128                 # 4 partition tiles over C
KH = (4 * C) // 128           # 16 partition tiles over 4C
SCALE = 1.0 / float(np.sqrt(D))
EPS = 1e-5
SCAL = 5.0                    # Chebyshev half-range in a-units (|a|max ~ 4.43)
KDEG = 12                     # Chebyshev series length (max err ~7e-3)
MNODES = 64                   # Chebyshev nodes per batch (2 batches -> 128 parts)
SS = SCALE / SCAL
IDF = 2 * KDEG + 1            # f32 identity offset inside constB
CLENSHAW_POOL = False          # batch-1 Clenshaw chain on GpSimd (Pool)

TRACE = False
TRACE_KW = {}
LAST_RESULTS = None

_CACHE = None


def _consts():
    m = np.arange(MNODES)
    theta = np.pi * (2 * m + 1) / (2 * MNODES)
    xn64 = (SCAL * np.cos(theta)).astype(np.float32)
    xnodes = np.concatenate([xn64, xn64])                 # [128] both batches
    dct1 = np.zeros((MNODES, KDEG), np.float32)
    for k in range(KDEG):
        dct1[:, k] = (2.0 / MNODES) * np.cos(k * theta)
    dct1[:, 0] *= 0.5
    dctbd = np.zeros((2 * MNODES, 2 * KDEG), np.float32)  # block-diag [128, 2K]
    dctbd[:MNODES, :KDEG] = dct1
    dctbd[MNODES:, KDEG:] = dct1
    # constA fp16 [128, 1]: ones/C column (LN stats stationary)
    constA = np.full((128, 1), 1.0 / C, np.float16)
    # constB f32: block-diag DCT | cheb nodes | f32 id | selKb | diagmask
    constB = np.zeros((128, IDF + 256 + KDEG), np.float32)
    constB[:, 0:2 * KDEG] = dctbd
    constB[:, 2 * KDEG] = xnodes
    constB[:, IDF:IDF + 128] = np.eye(128, dtype=np.float32)
    for r in range(2 * KDEG):
        for p in range(128):
            if r // KDEG == p // 64:
                constB[r, IDF + 128 + p] = 1.0
        constB[r, IDF + 256 + (r % KDEG)] = 1.0
    # constC fp16 [2, 640]: sel/halfs blocks | ones row
    constC = np.zeros((2, 640), np.float16)
    constC[0, 0:64] = 1.0
    constC[1, 64:128] = 1.0
    constC[:, 128:640] = 1.0
    return constA, constB, constC


def _build():
    nc = bacc.Bacc(debug=False, num_devices=NCORES)

    # ---- external inputs -------------------------------------------------
    seq_sl = nc.dram_tensor("seq_sl", [B, C, LC], FP16, kind="ExternalInput")
    exp16 = nc.dram_tensor("exp16", [B, G], FP16, kind="ExternalInput")
    wqt16 = nc.dram_tensor("wqt16", [C, C], FP16, kind="ExternalInput")     # Wq.T
    wot = nc.dram_tensor("wot", [C, C], FP16, kind="ExternalInput")         # (Wo*g2).T
    ut16d = nc.dram_tensor("ut16", [C, H], FP16, kind="ExternalInput")      # mask*uk
    us9d = nc.dram_tensor("us9d", [KC * (H + 1), 128], FP16, kind="ExternalInput")
    a0bd = nc.dram_tensor("a0bd", [128, H], FP16, kind="ExternalInput")     # SS*a0 bc
    w1t = nc.dram_tensor("w1t", [C, 4 * C], FP16, kind="ExternalInput")     # (W1*g1).T
    w2t = nc.dram_tensor("w2t", [4 * C, C], FP16, kind="ExternalInput")     # W2.T
    smallsf = nc.dram_tensor("smallsf", [32, 128], F32, kind="ExternalInput")

    out_sl = nc.dram_tensor("out_sl", [B, C, LC], F32, kind="ExternalOutput")

    constA_np, constB_np, constC_np = _consts()
    c_A = nc.inline_tensor(constA_np, name="c_A")
    c_B = nc.inline_tensor(constB_np, name="c_B")
    c_C = nc.inline_tensor(constC_np, name="c_C")

    with tile.TileContext(nc) as tc, ExitStack() as ctx:
        p_w1 = ctx.enter_context(tc.tile_pool(name="w1", bufs=KC))
        p_w2 = ctx.enter_context(tc.tile_pool(name="w2", bufs=16))
        p_kvh = ctx.enter_context(tc.tile_pool(name="kvh", bufs=8))
        p_wo = ctx.enter_context(tc.tile_pool(name="wo", bufs=KC))
        p_xsz = ctx.enter_context(tc.tile_pool(name="xsz", bufs=4))
        p_act = ctx.enter_context(tc.tile_pool(name="act", bufs=4))
        p_node = ctx.enter_context(tc.tile_pool(name="node", bufs=1))
        p_sm = ctx.enter_context(tc.tile_pool(name="sm", bufs=1))
        p_cl = ctx.enter_context(tc.tile_pool(name="cl", bufs=1))
        ps_mm = ctx.enter_context(tc.tile_pool(name="psmm", bufs=5, space="PSUM"))
        ps_x = ctx.enter_context(tc.tile_pool(name="psx", bufs=1, space="PSUM"))

        # ============ DMA queue programs (order == queue order) ===========
        # sync (SP): consts, stage-A host vectors, wqt, wot; w2 at the tail
        cC_sb = p_sm.tile([2, 640], FP16, tag="cC")
        nc.sync.dma_start(cC_sb[:], c_C[:])
        cA_sb = p_sm.tile([128, 1], FP16, tag="cA")
        nc.sync.dma_start(cA_sb[:], c_A[:])
        u_t = [p_sm.tile([128, H], FP16, tag=f"u{kt}", name=f"u_{kt}")
               for kt in range(KC)]
        for kt in range(KC):
            nc.sync.dma_start(u_t[kt][:], ut16d[kt * 128:(kt + 1) * 128, :])
        usel9 = [p_sm.tile([H + 1, 128], FP16, tag=f"us{kt}", name=f"us_{kt}")
                 for kt in range(KC)]
        for kt in range(KC):
            nc.sync.dma_start(usel9[kt][:],
                              us9d[kt * (H + 1):(kt + 1) * (H + 1), :])
        a0b = p_sm.tile([128, H], FP16, tag="a0b")
        nc.sync.dma_start(a0b[:], a0bd[:])
        wo_t = [p_wo.tile([128, C], FP16, tag="wo", name=f"wo_{i}")
                for i in range(KC)]
        for kt in range(KC):
            nc.sync.dma_start(wo_t[kt][:], wot[kt * 128:(kt + 1) * 128, :])

        # scalar (Act): exp only; bulk stays off this queue
        exp_sb = p_sm.tile([2, G], FP16, tag="expsb")
        nc.scalar.dma_start(exp_sb[:], exp16[:])

        # gpsimd (Pool): consts, xs, w1
        cB_sb = p_sm.tile([128, IDF + 256 + KDEG], F32, tag="cB")
        nc.gpsimd.dma_start(cB_sb[:], c_B[:])
        smf_sb = p_sm.tile([32, 128], F32, tag="smf")
        nc.gpsimd.dma_start(smf_sb[:], smallsf[:])
        xs_t = [p_xsz.tile([128, T], FP16, tag="xs", name=f"xs_{i}")
                for i in range(KC)]
        wqt_t = [p_kvh.tile([128, C], FP16, tag="kv", name=f"wqt_{i}")
                 for i in range(KC)]
        for kt in range(KC):
            srcx = seq_sl[:, kt * 128:(kt + 1) * 128, :].rearrange("b c l -> c b l")
            nc.gpsimd.dma_start(xs_t[kt][:], srcx)
            nc.gpsimd.dma_start(wqt_t[kt][:], wqt16[kt * 128:(kt + 1) * 128, :])
        w1_t = [p_w1.tile([128, 4 * C], FP16, tag="w1", name=f"w1_{i}")
                for i in range(KC)]
        for kt in range(KC):
            nc.gpsimd.dma_start(w1_t[kt][:], w1t[kt * 128:(kt + 1) * 128, :])

        eps_col = p_sm.tile([1, 1], F32, tag="epsc")
        nc.vector.memset(eps_col[:], EPS)

        # ============ PE: colsf transpose =================================
        colsf_ps = ps_x.tile([128, 32], F32, tag="small", bufs=1, name="colsfps")
        nc.tensor.transpose(colsf_ps[:], smf_sb[:], cB_sb[0:32, IDF:IDF + 32])
        colsf = p_sm.tile([128, 32], F32, tag="colsf")
        nc.vector.tensor_copy(colsf[:], colsf_ps[:])
        # colsf cols: bv 0:4 | g1 4:8 | b1p 8:24 | b2p 24:28 | bop 28:32

        # ============ PE: exp broadcast to [128, G] =======================
        eb_ps = [ps_mm.tile([128, 512], F32, tag="mm", name=f"ebps{j}")
                 for j in range(4)]
        for j in range(4):
            nc.tensor.matmul(eb_ps[j][:], cC_sb[:, 0:128],
                             exp_sb[:, j * 512:(j + 1) * 512],
                             start=True, stop=True)
        eb16 = p_node.tile([128, G], FP16, tag="eb16")
        for j in range(4):
            nc.scalar.copy(eb16[:, j * 512:(j + 1) * 512], eb_ps[j][:])

        # ============ node stage on Act: pn = exp(xn*e), accum z ==========
        pn = p_node.tile([128, G], F32, tag="pn")
        z4 = p_sm.tile([128, 4], F32, tag="z4")
        nm4 = p_sm.tile([128, 4], F32, tag="nm4")
        for j in range(4):
            nc.scalar.activation(pn[:, j * 512:(j + 1) * 512],
                                 eb16[:, j * 512:(j + 1) * 512], AF.Exp,
                                 scale=cB_sb[:, 2 * KDEG:2 * KDEG + 1],
                                 accum_out=z4[:, j:j + 1])

        # ============ DVE node tail: weighted sum, f ======================
        for j in range(4):
            nc.vector.scalar_tensor_tensor(
                out=pn[:, j * 512:(j + 1) * 512],
                in0=pn[:, j * 512:(j + 1) * 512], scalar=1.0,
                in1=eb16[:, j * 512:(j + 1) * 512],
                op0=OP.mult, op1=OP.mult, accum_out=nm4[:, j:j + 1])
        z_col = p_sm.tile([128, 1], F32, tag="zc")
        nc.vector.tensor_reduce(z_col[:], z4[:], axis=mybir.AxisListType.X, op=OP.add)
        nm_col = p_sm.tile([128, 1], F32, tag="nmc")
        nc.vector.tensor_reduce(nm_col[:], nm4[:], axis=mybir.AxisListType.X, op=OP.add)
        zr_col = p_sm.tile([128, 1], F32, tag="zrc")
        nc.vector.reciprocal(zr_col[:], z_col[:])
        f_col = p_sm.tile([128, 1], F32, tag="fc")
        nc.vector.tensor_mul(f_col[:], nm_col[:], zr_col[:])

        # ============ q = Wq.T-tiles @ xs (no M precompute) ===============
        # q[m, tau] = sum_c Wq[m, c] xs[c, tau]; stationary = Wq.T tiles
        q16 = []
        for mt in range(KC):
            q_ps = ps_mm.tile([128, T], F32, tag="mm", name=f"qps{mt}")
            for kt in range(KC):
                nc.tensor.matmul(q_ps[:], wqt_t[kt][:, mt * 128:(mt + 1) * 128],
                                 xs_t[kt][:], start=(kt == 0), stop=(kt == KC - 1))
            qm = p_act.tile([128, T], FP16, tag="q", name=f"q{mt}")
            nc.scalar.copy(qm[:], q_ps[:])
            q16.append(qm)
        # ============ coeffs: DCT matmul + DRAM round trip ================
        pck = ps_x.tile([2 * KDEG, 1], F32, tag="small", bufs=1, name="pck")
        nc.tensor.matmul(pck[:], cB_sb[:, 0:2 * KDEG], f_col[:],
                         start=True, stop=True)
        ck_sb = p_sm.tile([2 * KDEG, 1], F32, tag="cksb")
        nc.vector.tensor_copy(ck_sb[:], pck[:])
        ckdiag = p_sm.tile([2 * KDEG, KDEG], F32, tag="ckdiag")
        nc.vector.tensor_scalar_mul(
            ckdiag[:], cB_sb[0:2 * KDEG, IDF + 256:IDF + 256 + KDEG], ck_sb[:])
        cb_ps = ps_x.tile([128, KDEG], F32, tag="small", bufs=1, name="cbps")
        nc.tensor.matmul(cb_ps[:], cB_sb[0:2 * KDEG, IDF + 128:IDF + 256],
                         ckdiag[:], start=True, stop=True)
        cb = p_cl.tile([128, KDEG], F32, tag="cb")
        nc.vector.tensor_copy(cb[:], cb_ps[:])

        # ====== a.T packed [p = b*64+t64, (blk, h)] from q16 ==============
        FPK = 4 * (H + 1)                                 # 36 free cols
        HB = FPK // 2
        tt = p_cl.tile([128, FPK], F32, tag="tt")
        nc.gpsimd.memset(tt[:], 0.0)
        pa_all = ps_x.tile([128, 4 * H], F32, tag="st1", name="paall")
        for b in range(B):
            for blk in range(4):
                tsl = slice(b * 256 + blk * 64, b * 256 + (blk + 1) * 64)
                out = pa_all[b * 64:(b + 1) * 64, blk * 8:(blk + 1) * 8]
                for mt in range(KC):
                    nc.tensor.matmul(out, q16[mt][:, tsl], u_t[mt][:],
                                     start=(mt == 0), stop=(mt == KC - 1))
        for b in range(B):
            for blk in range(4):
                nc.vector.scalar_tensor_tensor(
                    out=tt[b * 64:(b + 1) * 64, blk * 9:blk * 9 + H],
                    in0=pa_all[b * 64:(b + 1) * 64, blk * 8:(blk + 1) * 8],
                    scalar=SS,
                    in1=a0b[b * 64:(b + 1) * 64, :],
                    op0=OP.mult, op1=OP.add)

        # ====== per-batch clamp + Clenshaw (DVE), unpack, y ===============
        tt2 = p_cl.tile([128, FPK], F32, tag="tt2")
        bb1 = p_cl.tile([128, FPK], F32, tag="bb1")
        bb2 = p_cl.tile([128, FPK], F32, tag="bb2")
        tmp = p_cl.tile([128, FPK], F32, tag="tmp")
        w_c = p_cl.tile([128, FPK], F32, tag="wp")
        w_rows = p_sm.tile([H + 1, T], FP16, tag="wrows")

        def clenshaw_full():
            nc.vector.tensor_scalar_max(tt[:], tt[:], -1.0)
            nc.vector.tensor_scalar_min(tt[:], tt[:], 1.0)
            nc.vector.tensor_add(tt2[:], tt[:], tt[:])
            cur1, cur2 = bb1, bb2
            for k in range(KDEG - 1, 0, -1):
                nc.vector.tensor_mul(tmp[:], tt2[:], cur1[:])
                nc.vector.scalar_tensor_tensor(
                    out=cur2[:], in0=tmp[:], scalar=cb[:, k:k + 1],
                    in1=cur2[:], op0=OP.add, op1=OP.subtract)
                cur1, cur2 = cur2, cur1
            nc.vector.tensor_mul(tmp[:], tt[:], cur1[:])
            nc.vector.scalar_tensor_tensor(
                out=w_c[:], in0=tmp[:], scalar=cb[:, 0:1],
                in1=cur2[:], op0=OP.add, op1=OP.subtract)
            for tb in range(4):
                nc.gpsimd.memset(w_c[:, tb * 9 + H:tb * 9 + H + 1], 1.0)

        def unpack_half(b):
            idsl = cB_sb[b * 64:(b + 1) * 64, IDF + b * 64:IDF + (b + 1) * 64]
            for blk in range(4):
                wr_ps = ps_x.tile([H + 1, 64], F32,
                                  tag="small" if blk % 2 == 0 else "st1",
                                  bufs=1, name=f"wrps{b}{blk}")
                nc.tensor.transpose(
                    wr_ps[:], w_c[b * 64:(b + 1) * 64, blk * 9:(blk + 1) * 9],
                    idsl)
                tsl = slice(b * 256 + blk * 64, b * 256 + (blk + 1) * 64)
                nc.scalar.copy(w_rows[0:H + 1, tsl], wr_ps[:])

        y_t = [p_act.tile([128, T], FP16, tag="y", bufs=8, name=f"y{kt}")
               for kt in range(KC)]

        def ymm_half(hf):
            hsl = slice(hf * 256, (hf + 1) * 256)
            for kt in range(KC):
                xa_ps = ps_mm.tile([128, 256], F32, tag="mm", name=f"xa{hf}{kt}")
                nc.tensor.matmul(xa_ps[:], usel9[kt][:], w_rows[:, hsl],
                                 start=True, stop=True)
                nc.vector.tensor_add(y_t[kt][:, hsl], xa_ps[:], xs_t[kt][:, hsl])

        # ==== token-half pipelined LN1 -> FFN1 -> FFN2 -> LN2 -> Wo -> out ====
        # Each half (256 tokens) flows independently; PE queue interleaves
        # halves so LN row-chains overlap the other half's matmuls.
        HT = T // 2

        def ln_stats(y_tiles, hf, ph):
            hsl = slice(hf * HT, (hf + 1) * HT)
            stat0 = ps_x.tile([1, HT], F32, tag="st0", name=f"st0{ph}{hf}")
            for kt in range(KC):
                nc.tensor.matmul(stat0[:], cA_sb[:, 0:1], y_tiles[kt][:, hsl],
                                 start=(kt == 0), stop=(kt == KC - 1))
            sq_t = []
            for kt in range(KC):
                sq = p_act.tile([128, HT], FP16, tag="sq", bufs=4,
                                name=f"sq{ph}{hf}{kt}")
                if kt % 2 == 0:
                    nc.scalar.activation(sq[:], y_tiles[kt][:, hsl], AF.Square)
                else:
                    nc.vector.tensor_mul(sq[:], y_tiles[kt][:, hsl],
                                         y_tiles[kt][:, hsl])
                sq_t.append(sq)
            musq = p_sm.tile([1, HT], F32, tag="lnrow", bufs=8, name=f"mu{ph}{hf}")
            nc.scalar.activation(musq[:], stat0[:], AF.Square)
            stat1 = ps_x.tile([1, HT], F32, tag="st1", name=f"st1{ph}{hf}")
            for kt in range(KC):
                nc.tensor.matmul(stat1[:], cA_sb[:, 0:1], sq_t[kt][:],
                                 start=(kt == 0), stop=(kt == KC - 1))
            vare = p_sm.tile([1, HT], F32, tag="lnrow", bufs=8, name=f"va{ph}{hf}")
            nc.vector.scalar_tensor_tensor(
                out=vare[:], in0=stat1[:], scalar=EPS, in1=musq[:],
                op0=OP.add, op1=OP.subtract)
            varr = p_sm.tile([1, HT], F32, tag="lnrow", bufs=8, name=f"vr{ph}{hf}")
            nc.vector.reciprocal(varr[:], vare[:])
            rstd_row = p_sm.tile([1, HT], FP16, tag="rstdr", bufs=4,
                                 name=f"rs{ph}{hf}")
            with nc.allow_low_precision(reason="fp16 feeds full-rate PE matmul"):
                nc.scalar.activation(rstd_row[:], varr[:], AF.Sqrt)
            q_row = p_sm.tile([1, HT], FP16, tag="qr", bufs=4, name=f"qq{ph}{hf}")
            nc.vector.tensor_mul(q_row[:], stat0[:], rstd_row[:])
            return rstd_row, q_row

        def ln_apply(y_tiles, rstd_row, q_row, out_tiles, hf, ph):
            hsl = slice(hf * HT, (hf + 1) * HT)
            pA = ps_mm.tile([128, HT], F32, tag="mm", name=f"pA{ph}{hf}")
            nc.tensor.matmul(pA[:], cC_sb[0:1, 128:256], rstd_row[:],
                             start=True, stop=True)
            pB = ps_mm.tile([128, HT], F32, tag="mm", name=f"pB{ph}{hf}")
            nc.tensor.matmul(pB[:], cC_sb[0:1, 128:256], q_row[:],
                             start=True, stop=True)
            for kt in range(KC):
                tx = p_act.tile([128, HT], FP16, tag="tmpx", bufs=4,
                                name=f"tx{ph}{hf}{kt}")
                eng = nc.vector if kt % 2 == 0 else nc.gpsimd
                nc.vector.tensor_mul(tx[:], y_tiles[kt][:, hsl], pA[:])
                nc.vector.tensor_sub(out_tiles[kt][:, hsl], tx[:], pB[:])

        n1_t = [p_act.tile([128, T], FP16, tag="x", name=f"n1{kt}")
                for kt in range(KC)]
        h_t = [p_kvh.tile([128, T], FP16, tag="h", bufs=16, name=f"h{mt}")
               for mt in range(KH)]
        y2_t = [p_act.tile([128, T], FP16, tag="y", bufs=8, name=f"y2{mt}")
                for mt in range(KC)]
        z_t = [p_xsz.tile([128, T], FP16, tag="z", name=f"z{mt}")
               for mt in range(KC)]

        def ffn1(hf):
            hsl = slice(hf * HT, (hf + 1) * HT)
            for mt in range(KH):
                sl = slice(mt * 128, (mt + 1) * 128)
                pf = ps_mm.tile([128, HT], F32, tag="mm", name=f"pf1{hf}{mt}")
                for kt in range(KC):
                    nc.tensor.matmul(pf[:], w1_t[kt][:, sl], n1_t[kt][:, hsl],
                                     start=(kt == 0), stop=(kt == KC - 1))
                if mt % 2 == 0:
                    nc.scalar.activation(h_t[mt][:, hsl], pf[:], AF.Relu,
                                         bias=colsf[:, 8 + mt:9 + mt])
                else:
                    nc.vector.tensor_scalar(h_t[mt][:, hsl], pf[:],
                                            colsf[:, 8 + mt:9 + mt],
                                            0.0, op0=OP.add, op1=OP.max)

        def ffn2(hf):
            hsl = slice(hf * HT, (hf + 1) * HT)
            for mt in range(KC):
                sl = slice(mt * 128, (mt + 1) * 128)
                pf = ps_mm.tile([128, HT], F32, tag="mm", name=f"pf2{hf}{mt}")
                for kt in range(KH):
                    nc.tensor.matmul(pf[:], w2_t[kt][:, sl], h_t[kt][:, hsl],
                                     start=(kt == 0), stop=(kt == KH - 1))
                tmp2 = p_act.tile([128, HT], FP16, tag="tmpx", bufs=4,
                                  name=f"t2{hf}{mt}")
                nc.scalar.activation(tmp2[:], pf[:], AF.Identity,
                                     bias=colsf[:, 24 + mt:25 + mt])
                nc.vector.scalar_tensor_tensor(
                    out=y2_t[mt][:, hsl], in0=n1_t[mt][:, hsl],
                    scalar=colsf[:, 4 + mt:5 + mt],
                    in1=tmp2[:], op0=OP.mult, op1=OP.add)

        def wo_out(hf):
            hsl = slice(hf * HT, (hf + 1) * HT)
            for mt in range(KC):
                sl = slice(mt * 128, (mt + 1) * 128)
                pf = ps_mm.tile([128, HT], F32, tag="mm", name=f"pfo{hf}{mt}")
                for kt in range(KC):
                    nc.tensor.matmul(pf[:], wo_t[kt][:, sl], z_t[kt][:, hsl],
                                     start=(kt == 0), stop=(kt == KC - 1))
                om = p_act.tile([128, HT], F32, tag="om", bufs=4,
                                name=f"om{hf}{mt}")
                nc.scalar.activation(om[:], pf[:], AF.Identity,
                                     bias=colsf[:, 28 + mt:29 + mt])
                # half hf covers tokens [hf*256, hf*256+256) = batch hf entirely
                eng = (nc.scalar, nc.sync, nc.gpsimd)[mt % 3]
                eng.dma_start(out_sl[hf, mt * 128:(mt + 1) * 128, :], om[:])

        # w2 loads (tail of sync queue; emitted late for sem-slot hygiene)
        w2_t = [p_w2.tile([128, C], FP16, tag="w2", name=f"w2_{i}")
                for i in range(KH)]
        for kt in range(KH):
            nc.sync.dma_start(w2_t[kt][:], w2t[kt * 128:(kt + 1) * 128, :])

        # interleaved schedule: batch-0 w-path flows while batch-1 Clenshaw
        # runs on DVE; halves alternate so LN row-chains hide under matmuls
        nc.gpsimd.memset(bb1[:], 0.0)
        nc.gpsimd.memset(bb2[:], 0.0)
        clenshaw_full()
        unpack_half(0)
        unpack_half(1)
        ymm_half(0)
        ymm_half(1)
        r0, q0 = ln_stats(y_t, 0, "a")
        r1, q1 = ln_stats(y_t, 1, "a")
        ln_apply(y_t, r0, q0, n1_t, 0, "a")
        ffn1(0)
        ln_apply(y_t, r1, q1, n1_t, 1, "a")
        ffn1(1)
        ffn2(0)
        r20, q20 = ln_stats(y2_t, 0, "b")
        ffn2(1)
        ln_apply(y2_t, r20, q20, z_t, 0, "b")
        r21, q21 = ln_stats(y2_t, 1, "b")
        wo_out(0)
        ln_apply(y2_t, r21, q21, z_t, 1, "b")
        wo_out(1)

    nc.compile()
    return nc


def kernel(**inputs):
    global _CACHE, LAST_RESULTS
    if _CACHE is None:
        _CACHE = _build()
    nc = _CACHE

    f32 = lambda x: np.ascontiguousarray(np.asarray(x), dtype=np.float32)
    f16 = lambda x: np.ascontiguousarray(np.asarray(x), dtype=np.float16)
    seq = f16(inputs["seq"])
    W1 = np.asarray(inputs["W1"], np.float32)
    W2 = np.asarray(inputs["W2"], np.float32)
    Wo = np.asarray(inputs["Wo"], np.float32)
    g1 = np.asarray(inputs["g1"], np.float32)
    g2 = np.asarray(inputs["g2"], np.float32)
    beta1 = np.asarray(inputs["beta1"], np.float32)
    beta2 = np.asarray(inputs["beta2"], np.float32)
    b1p = np.asarray(inputs["b1"], np.float32) + W1 @ beta1
    b2p = np.asarray(inputs["b2"], np.float32) + beta1
    bop = np.asarray(inputs["bo"], np.float32) + Wo @ beta2
    Wg = np.asarray(inputs["Wg"], np.float32)
    bg = np.asarray(inputs["bg"], np.float32)
    bq = np.asarray(inputs["bq"], np.float32)
    bv = np.asarray(inputs["bv"], np.float32)

    # stage-A vectors (token-independent, exact in fp32 on host)
    Wk = np.asarray(inputs["Wk"], np.float32)
    Wv = np.asarray(inputs["Wv"], np.float32)
    wg0 = Wg[:, 0]
    uk = Wk @ wg0                                     # [C]
    uv = Wv @ wg0                                     # [C]
    cv = Wv @ bg + bv                                 # [C]
    mask = np.zeros((C, H), np.float32)
    for h in range(H):
        mask[h * (C // H):(h + 1) * (C // H), h] = 1.0
    ut = mask * uk[:, None]                           # [C, H]
    a0 = ut.T @ bq                                    # [H]
    us9 = np.zeros((KC * (H + 1), 128), np.float32)
    for kt in range(KC):
        us9[kt * (H + 1):kt * (H + 1) + H, :] = \
            (mask[kt * 128:(kt + 1) * 128, :] * uv[kt * 128:(kt + 1) * 128, None]).T
        us9[kt * (H + 1) + H, :] = cv[kt * 128:(kt + 1) * 128]
    a0b = np.broadcast_to((SS * a0)[None, :], (128, H)).copy()
    smallsf = np.zeros((32, 128), np.float32)
    smallsf[0:4] = bv.reshape(4, 128)
    smallsf[4:8] = g1.reshape(4, 128)
    smallsf[8:24] = b1p.reshape(16, 128)
    smallsf[24:28] = b2p.reshape(4, 128)
    smallsf[28:32] = bop.reshape(4, 128)

    base = {
        "exp16": f16(inputs["exp"]),
        "wqt16": f16(np.asarray(inputs["Wq"]).T),
        "wot": f16((Wo * g2[None, :]).T),
        "ut16": f16(ut),
        "us9d": f16(us9),
        "a0bd": f16(a0b),
        "w1t": f16((W1 * g1[None, :]).T),
        "w2t": f16(W2.T),
        "smallsf": smallsf,
    }
    in_maps = []
    for c in range(NCORES):
        m = dict(base)
        m["seq_sl"] = np.ascontiguousarray(seq[:, :, c * LC:(c + 1) * LC])
        in_maps.append(m)

    res = run_bass_kernel_spmd(nc, in_maps, list(range(NCORES)), trace=TRACE,
                               **TRACE_KW)
    LAST_RESULTS = res
    out = np.empty((B, C, L), np.float32)
    for c in range(NCORES):
        out[:, :, c * LC:(c + 1) * LC] = res.results[c]["out_sl"]
    return out
